# revision 21
# baseline (speedup 1.0000x reference)
"""KNN entropy loss (k=5, B=8192, D=768) on 8 TRN2 NeuronCores.

Sharding: rows of x split 1024/core. Each core computes its [1024 x 8192]
block of v[i,j] = sum_{k<766} x_ik x_jk + (C - ||x_j||^2)/2 with fp8e4
DoubleRow matmuls (3 uniform pairs per 512-col chunk, f32 PSUM). The two
bias values ride as augmented contraction rows 766/767 (fp8 hi/lo split),
with 1.0 in those rows on the stationary side, so every chunk is exactly
3 equal-shape matmuls — no PE tile reconfiguration anywhere.

argmax_j v = argmin_j d^2, so a DVE MAX8 straight off each PSUM bank
yields per-chunk top-8 candidates; a second MAX8 merges the 16 chunk
results per row tile. The merged top-8 v values are DMA'd out; the host
reconstructs d = sqrt((||x_i||^2 + C) - 2 v) for ranks 1..5 (rank 0 is
the self-match), then loss = -mean(log(mean_k d + eps)).

All norms/layout prep happens on the host (like the baseline's
transpose/cast prep). Norms use the full 768 dims in f64; only the two
cross-term dims 766/767 of the gram are absorbed by the bias rows, which
perturbs d^2 by ~0.2% rms — far inside the 2e-2 tolerance (measured
rel err ~5e-5).
"""

import sys
import types

import numpy as np
import ml_dtypes

import concourse.bass as bass
import concourse.mybir as mybir
from concourse.tile import TileContext
from concourse.bass_utils import run_bass_kernel_spmd

P = 128
B = 8192
D = 768
NCORES = 8
BL = B // NCORES          # 1024 local rows per core
NPAIR = D // 256          # 3 DoubleRow contraction pairs
NI = BL // P              # 8 row tiles per core
NJ = B // 512             # 16 column chunks of 512
K = 5
EPS = 1e-8

FP8 = mybir.dt.float8e4
F32 = mybir.dt.float32
DR = mybir.MatmulPerfMode.DoubleRow


def _split_excess_waits(bir_json: bytes) -> bytes:
    """The walrus in this container rejects instructions carrying more than
    one sem-wait ("Too many sync wait commands"). Hoist all but the last
    wait of any instruction into single-wait EventSemaphore instructions
    inserted just before it on the same engine (same-engine program order
    makes this semantically identical)."""
    import json

    m = json.loads(bir_json)
    for f in m["functions"]:
        for bb in f["blocks"]:
            out_insts = []
            for ins in bb["instructions"]:
                si = ins.get("sync_info")
                waits = (si or {}).get("on_wait") or []
                if len(waits) > 1:
                    for i, w in enumerate(waits[:-1]):
                        out_insts.append(
                            {
                                "debug": ins.get("debug", 0),
                                "engine": ins["engine"],
                                "ins": [],
                                "name": f"{ins['name']}_sw{i}",
                                "opcode": "EventSemaphore",
                                "outs": [],
                                "sync_info": {"on_update": [], "on_wait": [w]},
                            }
                        )
                    si["on_wait"] = [waits[-1]]
                out_insts.append(ins)
            bb["instructions"] = out_insts
    return json.dumps(m).encode()


def _patch_compile_for_wait_limit():
    import concourse.bass_utils as bu
    import concourse.bass2jax as b2j

    if getattr(bu, "_wait_split_patched", False):
        return
    orig = bu.compile_bir_kernel

    def compile_bir_kernel(bir_json, tmpdir, neff_name="file.neff"):
        return orig(_split_excess_waits(bir_json), tmpdir, neff_name)

    bu.compile_bir_kernel = compile_bir_kernel
    b2j.compile_bir_kernel = compile_bir_kernel
    bu._wait_split_patched = True


def _install_ntff_hook_shim():
    """The trimmed image lacks antenv.axon_hooks; recreate it so
    run_bass_kernel_spmd(trace=True) can capture NTFF profiles via axon."""
    if "antenv.axon_hooks" in sys.modules:
        return
    try:
        import antenv
        from trn_agent_boot.trn_boot import _ntff_profile_via_ctypes
    except Exception:
        return
    mod = types.ModuleType("antenv.axon_hooks")
    _hook = _ntff_profile_via_ctypes("/opt/axon/libaxon_pjrt.so")
    mod.get_axon_ntff_profile_hook = lambda: _hook
    mod.set_axon_ntff_profile_hook = lambda h: None
    sys.modules["antenv.axon_hooks"] = mod
    antenv.axon_hooks = mod


def build_kernel() -> bass.Bass:
    nc = bass.Bass(target_bir_lowering=False, trn_type="TRN2")
    # full augmented x^T in DoubleRow pair layout: [pair, part, slab, col]
    xt = nc.dram_tensor("xt", [NPAIR, P, 2, B], FP8, kind="ExternalInput")
    # stationary side: same restricted to this core's rows, bias rows -> 1.0
    xtl = nc.dram_tensor("xtl", [NPAIR, P, 2, BL], FP8, kind="ExternalInput")
    out = nc.dram_tensor("out", [P, NI * 8], F32, kind="ExternalOutput")

    with TileContext(nc) as tc:
        with (
            tc.tile_pool(name="xtp", bufs=1) as xt_pool,
            tc.tile_pool(name="cnd", bufs=4) as cand_pool,
            tc.tile_pool(name="res", bufs=1) as res_pool,
            tc.tile_pool(name="ps", bufs=8, space="PSUM") as psum_pool,
        ):
            xtl_sb = []
            for pr in range(NPAIR):
                t = xt_pool.tile([P, 2, BL], FP8, name=f"xtl{pr}")
                xtl_sb.append(t)
            nc.gpsimd.dma_start(xtl_sb[0], xtl[0])
            nc.gpsimd.dma_start(xtl_sb[1], xtl[1])
            nc.gpsimd.dma_start(xtl_sb[2], xtl[2])

            # big moving tiles in column quarters; the first quarter of each
            # pair is issued from a different engine so the three issues (and
            # transfers) run concurrently and compute can start early
            xt_sb = [
                xt_pool.tile([P, 2, B], FP8, name=f"xt{pr}") for pr in range(NPAIR)
            ]
            NQD = 4
            H = B // NQD
            issue_engines = [nc.sync, nc.scalar, nc.gpsimd]
            for pr in range(NPAIR):
                issue_engines[pr].dma_start(
                    xt_sb[pr][:, :, 0:H], xt[pr][:, :, 0:H]
                )
            for h in range(1, NQD):
                for pr in range(NPAIR):
                    nc.sync.dma_start(
                        xt_sb[pr][:, :, h * H : (h + 1) * H],
                        xt[pr][:, :, h * H : (h + 1) * H],
                    )

            # PE clock warmup during the DMA: cheap 16-col-stationary matmuls
            # (one tile reconfig before the main sweep, none inside it)
            wu = psum_pool.tile([P, 512], F32, name="ps")
            for w in range(16):
                nc.tensor.matmul(
                    wu[:, 0:8],
                    lhsT=xtl_sb[0][:, :, 0:P],
                    rhs=xtl_sb[0][:, :, 0:8],
                    start=True,
                    stop=True,
                    perf_mode=DR,
                )

            # ---- main sweep ----
            # Chunks 0-7 of each row tile: DVE MAX8 straight off the PSUM
            # bank. Chunks 8-15: scalar evacuates the bank to an SBUF strip;
            # DVE then takes top-8 of each 4-chunk strip in one MAX8. This
            # balances DVE (~10.1us/row tile) against PE (~10.4us). The last
            # row tile stays all-direct so the tail ends with a short MAX8.
            top_all = res_pool.tile([P, NI * 8], F32, name="top_all")
            cands = {}
            NDIR = 8  # chunks per row tile handled directly from PSUM
            for i in range(NI):
                split = i < NI - 1
                cand = cand_pool.tile([P, NJ * 8], F32, name="cand")
                cands[i] = cand
                nslots = 10 if split else NJ
                ms = [
                    cand_pool.tile([P, 4 * 512], F32, name=f"m{t}")
                    for t in range(2)
                ]
                for j in range(NJ):
                    ps = psum_pool.tile([P, 512], F32, name="ps")
                    for pr in range(NPAIR):
                        nc.tensor.matmul(
                            ps,
                            lhsT=xtl_sb[pr][:, :, i * P : (i + 1) * P],
                            rhs=xt_sb[pr][:, :, j * 512 : (j + 1) * 512],
                            start=(pr == 0),
                            stop=(pr == NPAIR - 1),
                            perf_mode=DR,
                        )
                    if not split or j < NDIR:
                        nc.vector.max(out=cand[:, j * 8 : (j + 1) * 8], in_=ps)
                    else:
                        t = (j - NDIR) // 4
                        s = (j - NDIR) % 4
                        nc.scalar.copy(ms[t][:, s * 512 : (s + 1) * 512], ps)
                        if s == 3:
                            nc.vector.max(
                                out=cand[:, (NDIR + t) * 8 : (NDIR + t + 1) * 8],
                                in_=ms[t],
                            )
                    # deferred merge of the previous row tile
                    if i > 0 and j == 2:
                        nc.vector.max(
                            out=top_all[:, (i - 1) * 8 : i * 8],
                            in_=cands[i - 1][:, 0 : 10 * 8],
                        )
            nc.vector.max(
                out=top_all[:, (NI - 1) * 8 :], in_=cands[NI - 1][:, 0 : NJ * 8]
            )
            nc.sync.dma_start(out[:], top_all)

    return nc


def _prep_inputs(x: np.ndarray):
    f8 = ml_dtypes.float8_e4m3
    sq = np.sum(x.astype(np.float64) * x.astype(np.float64), axis=1)
    C = float(sq.mean())
    bias_full = ((C - sq) / 2.0).astype(np.float32)
    bias_hi = bias_full.astype(f8)
    bias_lo = (bias_full - bias_hi.astype(np.float32)).astype(f8)
    # augmented moving operand: rows 0..765 = x dims, 766/767 = bias hi/lo
    xaug = np.empty((D, B), f8)
    xaug[: D - 2] = x.T[: D - 2].astype(f8)
    xaug[D - 2] = bias_hi
    xaug[D - 1] = bias_lo
    # stationary variant: bias rows replaced by 1.0
    xaug_st = xaug.copy()
    xaug_st[D - 2 :] = np.float32(1.0).astype(f8)
    # [768, 8192] -> [pair, part, slab, col]
    xt_dr = np.ascontiguousarray(
        xaug.reshape(NPAIR, 2, P, B).transpose(0, 2, 1, 3)
    )
    xt_dr_st = np.ascontiguousarray(
        xaug_st.reshape(NPAIR, 2, P, B).transpose(0, 2, 1, 3)
    )
    sq32 = (sq + C).astype(np.float32)
    return xt_dr, xt_dr_st, sq32


def run(inputs: dict, trace: bool = False):
    _patch_compile_for_wait_limit()
    if trace:
        _install_ntff_hook_shim()

    x = np.asarray(inputs["student_output"], dtype=np.float32)
    assert x.shape == (B, D), x.shape
    xt_dr, xt_dr_st, sq32 = _prep_inputs(x)

    nc = build_kernel()
    in_maps = []
    for c in range(NCORES):
        r0 = c * BL
        in_maps.append(
            {
                "xt": xt_dr,
                "xtl": np.ascontiguousarray(xt_dr_st[:, :, :, r0 : r0 + BL]),
            }
        )
    res = run_bass_kernel_spmd(
        nc, in_maps, core_ids=list(range(NCORES)), trace=trace
    )
    # host epilogue: top8 v values -> distances -> loss
    total = 0.0
    for c in range(NCORES):
        top8 = res.results[c]["out"].astype(np.float64)  # [P, NI*8]
        v5 = top8.reshape(P, NI, 8)[:, :, 1 : K + 1]     # drop self, keep 5 NN
        sqc = sq32[c * BL : (c + 1) * BL].reshape(NI, P).T  # [P, NI]
        d5 = np.sqrt(np.maximum(sqc[:, :, None] - 2.0 * v5, 0.0))
        total += np.log(d5.mean(axis=2) + EPS).sum()
    loss = np.float32(-total / B)
    return np.asarray(loss, dtype=np.float32), res


def kernel(**inputs) -> np.ndarray:
    out, _ = run(inputs, trace=False)
    return out


# revision 23
# speedup vs baseline: 1.0243x; 1.0243x over previous
"""KNN entropy loss (k=5, B=8192, D=768) on 8 TRN2 NeuronCores.

Sharding: rows of x split 1024/core. Each core computes its [1024 x 8192]
block of v[i,j] = sum_{k<766} x_ik x_jk + (C - ||x_j||^2)/2 with fp8e4
DoubleRow matmuls (3 uniform pairs per 512-col chunk, f32 PSUM). The two
bias values ride as augmented contraction rows 766/767 (fp8 hi/lo split),
with 1.0 in those rows on the stationary side, so every chunk is exactly
3 equal-shape matmuls — no PE tile reconfiguration anywhere.

argmax_j v = argmin_j d^2, so a DVE MAX8 straight off each PSUM bank
yields per-chunk top-8 candidates; a second MAX8 merges the 16 chunk
results per row tile. The merged top-8 v values are DMA'd out; the host
reconstructs d = sqrt((||x_i||^2 + C) - 2 v) for ranks 1..5 (rank 0 is
the self-match), then loss = -mean(log(mean_k d + eps)).

All norms/layout prep happens on the host (like the baseline's
transpose/cast prep). Norms use the full 768 dims in f64; only the two
cross-term dims 766/767 of the gram are absorbed by the bias rows, which
perturbs d^2 by ~0.2% rms — far inside the 2e-2 tolerance (measured
rel err ~5e-5).
"""

import sys
import types

import numpy as np
import ml_dtypes

import concourse.bass as bass
import concourse.mybir as mybir
from concourse.tile import TileContext
from concourse.bass_utils import run_bass_kernel_spmd

P = 128
B = 8192
D = 768
NCORES = 8
BL = B // NCORES          # 1024 local rows per core
NPAIR = D // 256          # 3 DoubleRow contraction pairs
NI = BL // P              # 8 row tiles per core
NJ = B // 512             # 16 column chunks of 512
K = 5
EPS = 1e-8

FP8 = mybir.dt.float8e4
F32 = mybir.dt.float32
DR = mybir.MatmulPerfMode.DoubleRow


def _split_excess_waits(bir_json: bytes) -> bytes:
    """The walrus in this container rejects instructions carrying more than
    one sem-wait ("Too many sync wait commands"). Hoist all but the last
    wait of any instruction into single-wait EventSemaphore instructions
    inserted just before it on the same engine (same-engine program order
    makes this semantically identical)."""
    import json

    m = json.loads(bir_json)
    for f in m["functions"]:
        for bb in f["blocks"]:
            out_insts = []
            for ins in bb["instructions"]:
                si = ins.get("sync_info")
                waits = (si or {}).get("on_wait") or []
                if len(waits) > 1:
                    for i, w in enumerate(waits[:-1]):
                        out_insts.append(
                            {
                                "debug": ins.get("debug", 0),
                                "engine": ins["engine"],
                                "ins": [],
                                "name": f"{ins['name']}_sw{i}",
                                "opcode": "EventSemaphore",
                                "outs": [],
                                "sync_info": {"on_update": [], "on_wait": [w]},
                            }
                        )
                    si["on_wait"] = [waits[-1]]
                out_insts.append(ins)
            bb["instructions"] = out_insts
    return json.dumps(m).encode()


def _patch_compile_for_wait_limit():
    import concourse.bass_utils as bu
    import concourse.bass2jax as b2j

    if getattr(bu, "_wait_split_patched", False):
        return
    orig = bu.compile_bir_kernel

    def compile_bir_kernel(bir_json, tmpdir, neff_name="file.neff"):
        return orig(_split_excess_waits(bir_json), tmpdir, neff_name)

    bu.compile_bir_kernel = compile_bir_kernel
    b2j.compile_bir_kernel = compile_bir_kernel
    bu._wait_split_patched = True


def _install_ntff_hook_shim():
    """The trimmed image lacks antenv.axon_hooks; recreate it so
    run_bass_kernel_spmd(trace=True) can capture NTFF profiles via axon."""
    if "antenv.axon_hooks" in sys.modules:
        return
    try:
        import antenv
        from trn_agent_boot.trn_boot import _ntff_profile_via_ctypes
    except Exception:
        return
    mod = types.ModuleType("antenv.axon_hooks")
    _hook = _ntff_profile_via_ctypes("/opt/axon/libaxon_pjrt.so")
    mod.get_axon_ntff_profile_hook = lambda: _hook
    mod.set_axon_ntff_profile_hook = lambda h: None
    sys.modules["antenv.axon_hooks"] = mod
    antenv.axon_hooks = mod


def build_kernel() -> bass.Bass:
    nc = bass.Bass(target_bir_lowering=False, trn_type="TRN2")
    # full augmented x^T in DoubleRow pair layout: [pair, part, slab, col]
    xt = nc.dram_tensor("xt", [NPAIR, P, 2, B], FP8, kind="ExternalInput")
    # stationary side: same restricted to this core's rows, bias rows -> 1.0
    xtl = nc.dram_tensor("xtl", [NPAIR, P, 2, BL], FP8, kind="ExternalInput")
    out = nc.dram_tensor("out", [P, NI * 8], F32, kind="ExternalOutput")

    with TileContext(nc) as tc:
        with (
            tc.tile_pool(name="xtp", bufs=1) as xt_pool,
            tc.tile_pool(name="cnd", bufs=2) as cand_pool,
            tc.tile_pool(name="res", bufs=1) as res_pool,
            tc.tile_pool(name="ps", bufs=8, space="PSUM") as psum_pool,
        ):
            xtl_sb = []
            for pr in range(NPAIR):
                t = xt_pool.tile([P, 2, BL], FP8, name=f"xtl{pr}")
                xtl_sb.append(t)
            nc.gpsimd.dma_start(xtl_sb[0], xtl[0])
            nc.gpsimd.dma_start(xtl_sb[1], xtl[1])
            nc.gpsimd.dma_start(xtl_sb[2], xtl[2])

            # big moving tiles in column quarters; the first quarter of each
            # pair is issued from a different engine so the three issues (and
            # transfers) run concurrently and compute can start early
            xt_sb = [
                xt_pool.tile([P, 2, B], FP8, name=f"xt{pr}") for pr in range(NPAIR)
            ]
            NQD = 4
            H = B // NQD
            issue_engines = [nc.sync, nc.scalar, nc.gpsimd]
            for pr in range(NPAIR):
                issue_engines[pr].dma_start(
                    xt_sb[pr][:, :, 0:H], xt[pr][:, :, 0:H]
                )
            for h in range(1, NQD):
                for pr in range(NPAIR):
                    nc.sync.dma_start(
                        xt_sb[pr][:, :, h * H : (h + 1) * H],
                        xt[pr][:, :, h * H : (h + 1) * H],
                    )

            # PE clock warmup during the DMA: cheap 16-col-stationary matmuls
            # (one tile reconfig before the main sweep, none inside it)
            wu = psum_pool.tile([P, 512], F32, name="ps")
            for w in range(16):
                nc.tensor.matmul(
                    wu[:, 0:8],
                    lhsT=xtl_sb[0][:, :, 0:P],
                    rhs=xtl_sb[0][:, :, 0:8],
                    start=True,
                    stop=True,
                    perf_mode=DR,
                )

            # ---- main sweep ----
            # Chunks 0-7 of each row tile: DVE MAX8 straight off the PSUM
            # bank. Chunks 8-15: scalar evacuates the bank to an SBUF strip;
            # DVE then takes top-8 of each 4-chunk strip in one MAX8. This
            # balances DVE (~10.1us/row tile) against PE (~10.4us). The last
            # row tile stays all-direct so the tail ends with a short MAX8.
            top_all = res_pool.tile([P, NI * 8], F32, name="top_all")
            cands = {}
            for i in range(NI):
                cand = cand_pool.tile([P, NJ * 8], F32, name="cand")
                cands[i] = cand
                for j in range(NJ):
                    ps = psum_pool.tile([P, 512], F32, name="ps")
                    for pr in range(NPAIR):
                        nc.tensor.matmul(
                            ps,
                            lhsT=xtl_sb[pr][:, :, i * P : (i + 1) * P],
                            rhs=xt_sb[pr][:, :, j * 512 : (j + 1) * 512],
                            start=(pr == 0),
                            stop=(pr == NPAIR - 1),
                            perf_mode=DR,
                        )
                    nc.vector.max(out=cand[:, j * 8 : (j + 1) * 8], in_=ps)
                    # deferred merge of the previous row tile so it doesn't
                    # sit between chunk maxes and delay PSUM bank recycling
                    if i > 0 and j == 2:
                        nc.vector.max(
                            out=top_all[:, (i - 1) * 8 : i * 8], in_=cands[i - 1]
                        )
            nc.vector.max(out=top_all[:, (NI - 1) * 8 :], in_=cands[NI - 1])
            nc.sync.dma_start(out[:], top_all)

    return nc


def _prep_inputs(x: np.ndarray):
    f8 = ml_dtypes.float8_e4m3
    sq = np.sum(x.astype(np.float64) * x.astype(np.float64), axis=1)
    C = float(sq.mean())
    bias_full = ((C - sq) / 2.0).astype(np.float32)
    bias_hi = bias_full.astype(f8)
    bias_lo = (bias_full - bias_hi.astype(np.float32)).astype(f8)
    # augmented moving operand: rows 0..765 = x dims, 766/767 = bias hi/lo
    xaug = np.empty((D, B), f8)
    xaug[: D - 2] = x.T[: D - 2].astype(f8)
    xaug[D - 2] = bias_hi
    xaug[D - 1] = bias_lo
    # stationary variant: bias rows replaced by 1.0
    xaug_st = xaug.copy()
    xaug_st[D - 2 :] = np.float32(1.0).astype(f8)
    # [768, 8192] -> [pair, part, slab, col]
    xt_dr = np.ascontiguousarray(
        xaug.reshape(NPAIR, 2, P, B).transpose(0, 2, 1, 3)
    )
    xt_dr_st = np.ascontiguousarray(
        xaug_st.reshape(NPAIR, 2, P, B).transpose(0, 2, 1, 3)
    )
    sq32 = (sq + C).astype(np.float32)
    return xt_dr, xt_dr_st, sq32


def run(inputs: dict, trace: bool = False):
    _patch_compile_for_wait_limit()
    if trace:
        _install_ntff_hook_shim()

    x = np.asarray(inputs["student_output"], dtype=np.float32)
    assert x.shape == (B, D), x.shape
    xt_dr, xt_dr_st, sq32 = _prep_inputs(x)

    nc = build_kernel()
    in_maps = []
    for c in range(NCORES):
        r0 = c * BL
        in_maps.append(
            {
                "xt": xt_dr,
                "xtl": np.ascontiguousarray(xt_dr_st[:, :, :, r0 : r0 + BL]),
            }
        )
    res = run_bass_kernel_spmd(
        nc, in_maps, core_ids=list(range(NCORES)), trace=trace
    )
    # host epilogue: top8 v values -> distances -> loss
    total = 0.0
    for c in range(NCORES):
        top8 = res.results[c]["out"].astype(np.float64)  # [P, NI*8]
        v5 = top8.reshape(P, NI, 8)[:, :, 1 : K + 1]     # drop self, keep 5 NN
        sqc = sq32[c * BL : (c + 1) * BL].reshape(NI, P).T  # [P, NI]
        d5 = np.sqrt(np.maximum(sqc[:, :, None] - 2.0 * v5, 0.0))
        total += np.log(d5.mean(axis=2) + EPS).sum()
    loss = np.float32(-total / B)
    return np.asarray(loss, dtype=np.float32), res


def kernel(**inputs) -> np.ndarray:
    out, _ = run(inputs, trace=False)
    return out


# revision 25
# speedup vs baseline: 1.0276x; 1.0033x over previous
"""KNN entropy loss (k=5, B=8192, D=768) on 8 TRN2 NeuronCores.

Sharding: rows of x split 1024/core. Each core computes its [1024 x 8192]
block of v[i,j] = sum_{k<766} x_ik x_jk + (C - ||x_j||^2)/2 with fp8e4
DoubleRow matmuls (3 uniform pairs per 512-col chunk, f32 PSUM). The two
bias values ride as augmented contraction rows 766/767 (fp8 hi/lo split),
with 1.0 in those rows on the stationary side, so every chunk is exactly
3 equal-shape matmuls — no PE tile reconfiguration anywhere.

argmax_j v = argmin_j d^2, so a DVE MAX8 straight off each PSUM bank
yields per-chunk top-8 candidates; a second MAX8 merges the 16 chunk
results per row tile. The merged top-8 v values are DMA'd out; the host
reconstructs d = sqrt((||x_i||^2 + C) - 2 v) for ranks 1..5 (rank 0 is
the self-match), then loss = -mean(log(mean_k d + eps)).

All norms/layout prep happens on the host (like the baseline's
transpose/cast prep). Norms use the full 768 dims in f64; only the two
cross-term dims 766/767 of the gram are absorbed by the bias rows, which
perturbs d^2 by ~0.2% rms — far inside the 2e-2 tolerance (measured
rel err ~5e-5).
"""

import sys
import types

import numpy as np
import ml_dtypes

import concourse.bass as bass
import concourse.mybir as mybir
from concourse.tile import TileContext
from concourse.bass_utils import run_bass_kernel_spmd

P = 128
B = 8192
D = 768
NCORES = 8
BL = B // NCORES          # 1024 local rows per core
NPAIR = D // 256          # 3 DoubleRow contraction pairs
NI = BL // P              # 8 row tiles per core
NJ = B // 512             # 16 column chunks of 512
K = 5
EPS = 1e-8

FP8 = mybir.dt.float8e4
F32 = mybir.dt.float32
DR = mybir.MatmulPerfMode.DoubleRow


def _split_excess_waits(bir_json: bytes) -> bytes:
    """The walrus in this container rejects instructions carrying more than
    one sem-wait ("Too many sync wait commands"). Hoist all but the last
    wait of any instruction into single-wait EventSemaphore instructions
    inserted just before it on the same engine (same-engine program order
    makes this semantically identical)."""
    import json

    m = json.loads(bir_json)
    for f in m["functions"]:
        for bb in f["blocks"]:
            out_insts = []
            for ins in bb["instructions"]:
                si = ins.get("sync_info")
                waits = (si or {}).get("on_wait") or []
                if len(waits) > 1:
                    for i, w in enumerate(waits[:-1]):
                        out_insts.append(
                            {
                                "debug": ins.get("debug", 0),
                                "engine": ins["engine"],
                                "ins": [],
                                "name": f"{ins['name']}_sw{i}",
                                "opcode": "EventSemaphore",
                                "outs": [],
                                "sync_info": {"on_update": [], "on_wait": [w]},
                            }
                        )
                    si["on_wait"] = [waits[-1]]
                out_insts.append(ins)
            bb["instructions"] = out_insts
    return json.dumps(m).encode()


def _patch_compile_for_wait_limit():
    import concourse.bass_utils as bu
    import concourse.bass2jax as b2j

    if getattr(bu, "_wait_split_patched", False):
        return
    orig = bu.compile_bir_kernel

    def compile_bir_kernel(bir_json, tmpdir, neff_name="file.neff"):
        return orig(_split_excess_waits(bir_json), tmpdir, neff_name)

    bu.compile_bir_kernel = compile_bir_kernel
    b2j.compile_bir_kernel = compile_bir_kernel
    bu._wait_split_patched = True


def _install_ntff_hook_shim():
    """The trimmed image lacks antenv.axon_hooks; recreate it so
    run_bass_kernel_spmd(trace=True) can capture NTFF profiles via axon."""
    if "antenv.axon_hooks" in sys.modules:
        return
    try:
        import antenv
        from trn_agent_boot.trn_boot import _ntff_profile_via_ctypes
    except Exception:
        return
    mod = types.ModuleType("antenv.axon_hooks")
    _hook = _ntff_profile_via_ctypes("/opt/axon/libaxon_pjrt.so")
    mod.get_axon_ntff_profile_hook = lambda: _hook
    mod.set_axon_ntff_profile_hook = lambda h: None
    sys.modules["antenv.axon_hooks"] = mod
    antenv.axon_hooks = mod


def build_kernel() -> bass.Bass:
    nc = bass.Bass(target_bir_lowering=False, trn_type="TRN2")
    # full augmented x^T in DoubleRow pair layout: [pair, part, slab, col]
    xt = nc.dram_tensor("xt", [NPAIR, P, 2, B], FP8, kind="ExternalInput")
    # stationary side: same restricted to this core's rows, bias rows -> 1.0
    xtl = nc.dram_tensor("xtl", [NPAIR, P, 2, BL], FP8, kind="ExternalInput")
    out = nc.dram_tensor("out", [P, NI * 8], F32, kind="ExternalOutput")

    with TileContext(nc) as tc:
        with (
            tc.tile_pool(name="xtp", bufs=1) as xt_pool,
            tc.tile_pool(name="cnd", bufs=2) as cand_pool,
            tc.tile_pool(name="res", bufs=1) as res_pool,
            tc.tile_pool(name="ps", bufs=8, space="PSUM") as psum_pool,
        ):
            xtl_sb = []
            for pr in range(NPAIR):
                t = xt_pool.tile([P, 2, BL], FP8, name=f"xtl{pr}")
                xtl_sb.append(t)
            nc.gpsimd.dma_start(xtl_sb[0], xtl[0])
            nc.gpsimd.dma_start(xtl_sb[1], xtl[1])
            nc.gpsimd.dma_start(xtl_sb[2], xtl[2])

            # big moving tiles in column quarters; the first quarter of each
            # pair is issued from a different engine so the three issues (and
            # transfers) run concurrently and compute can start early
            xt_sb = [
                xt_pool.tile([P, 2, B], FP8, name=f"xt{pr}") for pr in range(NPAIR)
            ]
            NQD = 4
            H = B // NQD
            issue_engines = [nc.sync, nc.scalar, nc.gpsimd]
            for pr in range(NPAIR):
                issue_engines[pr].dma_start(
                    xt_sb[pr][:, :, 0:H], xt[pr][:, :, 0:H]
                )
            for h in range(1, NQD):
                for pr in range(NPAIR):
                    nc.sync.dma_start(
                        xt_sb[pr][:, :, h * H : (h + 1) * H],
                        xt[pr][:, :, h * H : (h + 1) * H],
                    )

            # PE clock warmup during the DMA: cheap 16-col-stationary matmuls
            # (one tile reconfig before the main sweep, none inside it)
            wu = psum_pool.tile([P, 512], F32, name="ps")
            for w in range(16):
                nc.tensor.matmul(
                    wu[:, 0:8],
                    lhsT=xtl_sb[0][:, :, 0:P],
                    rhs=xtl_sb[0][:, :, 0:8],
                    start=True,
                    stop=True,
                    perf_mode=DR,
                )

            # ---- main sweep ----
            # Chunks 0-7 of each row tile: DVE MAX8 straight off the PSUM
            # bank. Chunks 8-15: scalar evacuates the bank to an SBUF strip;
            # DVE then takes top-8 of each 4-chunk strip in one MAX8. This
            # balances DVE (~10.1us/row tile) against PE (~10.4us). The last
            # row tile stays all-direct so the tail ends with a short MAX8.
            top_all = res_pool.tile([P, NI * 8], F32, name="top_all")
            cands = {}
            for i in range(NI):
                cand = cand_pool.tile([P, NJ * 8], F32, name="cand")
                cands[i] = cand
                for j in range(NJ):
                    ps = psum_pool.tile([P, 512], F32, name="ps")
                    for pr in range(NPAIR):
                        nc.tensor.matmul(
                            ps,
                            lhsT=xtl_sb[pr][:, :, i * P : (i + 1) * P],
                            rhs=xt_sb[pr][:, :, j * 512 : (j + 1) * 512],
                            start=(pr == 0),
                            stop=(pr == NPAIR - 1),
                            perf_mode=DR,
                        )
                    nc.vector.max(out=cand[:, j * 8 : (j + 1) * 8], in_=ps)
                    # deferred merge of the previous row tile so it doesn't
                    # sit between chunk maxes and delay PSUM bank recycling
                    if i > 0 and j == 2:
                        nc.vector.max(
                            out=top_all[:, (i - 1) * 8 : i * 8], in_=cands[i - 1]
                        )
            nc.vector.max(out=top_all[:, (NI - 1) * 8 :], in_=cands[NI - 1])
            nc.sync.dma_start(out[:], top_all)

    return nc


def _prep_inputs(x: np.ndarray):
    f8 = ml_dtypes.float8_e4m3
    sq = np.sum(x.astype(np.float64) * x.astype(np.float64), axis=1)
    C = float(sq.mean())
    bias_full = ((C - sq) / 2.0).astype(np.float32)
    bias_hi = bias_full.astype(f8)
    bias_lo = (bias_full - bias_hi.astype(np.float32)).astype(f8)
    # augmented moving operand: rows 0..765 = x dims, 766/767 = bias hi/lo
    xaug = np.empty((D, B), f8)
    xaug[: D - 2] = x.T[: D - 2].astype(f8)
    xaug[D - 2] = bias_hi
    xaug[D - 1] = bias_lo
    # stationary variant: bias rows replaced by 1.0
    xaug_st = xaug.copy()
    xaug_st[D - 2 :] = np.float32(1.0).astype(f8)
    # [768, 8192] -> [pair, part, slab, col]
    xt_dr = np.ascontiguousarray(
        xaug.reshape(NPAIR, 2, P, B).transpose(0, 2, 1, 3)
    )
    xt_dr_st = np.ascontiguousarray(
        xaug_st.reshape(NPAIR, 2, P, B).transpose(0, 2, 1, 3)
    )
    sq32 = (sq + C).astype(np.float32)
    return xt_dr, xt_dr_st, sq32


def run(inputs: dict, trace: bool = False):
    _patch_compile_for_wait_limit()
    if trace:
        _install_ntff_hook_shim()

    x = np.asarray(inputs["student_output"], dtype=np.float32)
    assert x.shape == (B, D), x.shape
    xt_dr, xt_dr_st, sq32 = _prep_inputs(x)

    nc = build_kernel()
    in_maps = []
    for c in range(NCORES):
        r0 = c * BL
        in_maps.append(
            {
                "xt": xt_dr,
                "xtl": np.ascontiguousarray(xt_dr_st[:, :, :, r0 : r0 + BL]),
            }
        )
    res = run_bass_kernel_spmd(
        nc, in_maps, core_ids=list(range(NCORES)), trace=trace
    )
    # host epilogue: top8 v values -> distances -> loss
    total = 0.0
    for c in range(NCORES):
        top8 = res.results[c]["out"].astype(np.float64)  # [P, NI*8]
        v5 = top8.reshape(P, NI, 8)[:, :, 1 : K + 1]     # drop self, keep 5 NN
        sqc = sq32[c * BL : (c + 1) * BL].reshape(NI, P).T  # [P, NI]
        d5 = np.sqrt(np.maximum(sqc[:, :, None] - 2.0 * v5, 0.0))
        total += np.log(d5.mean(axis=2) + EPS).sum()
    loss = np.float32(-total / B)
    return np.asarray(loss, dtype=np.float32), res


def kernel(**inputs) -> np.ndarray:
    out, _ = run(inputs, trace=False)
    return out


# revision 26
# speedup vs baseline: 1.0348x; 1.0070x over previous
"""KNN entropy loss (k=5, B=8192, D=768) on 8 TRN2 NeuronCores.

Sharding: rows of x split 1024/core. Each core computes its [1024 x 8192]
block of v[i,j] = sum_{k<766} x_ik x_jk + (C - ||x_j||^2)/2 with fp8e4
DoubleRow matmuls (3 uniform pairs per 512-col chunk, f32 PSUM). The two
bias values ride as augmented contraction rows 766/767 (fp8 hi/lo split),
with 1.0 in those rows on the stationary side, so every chunk is exactly
3 equal-shape matmuls — no PE tile reconfiguration anywhere.

argmax_j v = argmin_j d^2, so a DVE MAX8 straight off each PSUM bank
yields per-chunk top-8 candidates; a second MAX8 merges the 16 chunk
results per row tile. The merged top-8 v values are DMA'd out; the host
reconstructs d = sqrt((||x_i||^2 + C) - 2 v) for ranks 1..5 (rank 0 is
the self-match), then loss = -mean(log(mean_k d + eps)).

All norms/layout prep happens on the host (like the baseline's
transpose/cast prep). Norms use the full 768 dims in f64; only the two
cross-term dims 766/767 of the gram are absorbed by the bias rows, which
perturbs d^2 by ~0.2% rms — far inside the 2e-2 tolerance (measured
rel err ~5e-5).
"""

import sys
import types

import numpy as np
import ml_dtypes

import concourse.bass as bass
import concourse.mybir as mybir
from concourse.tile import TileContext
from concourse.bass_utils import run_bass_kernel_spmd

P = 128
B = 8192
D = 768
NCORES = 8
BL = B // NCORES          # 1024 local rows per core
NPAIR = D // 256          # 3 DoubleRow contraction pairs
NI = BL // P              # 8 row tiles per core
NJ = B // 512             # 16 column chunks of 512
K = 5
EPS = 1e-8

FP8 = mybir.dt.float8e4
F32 = mybir.dt.float32
DR = mybir.MatmulPerfMode.DoubleRow


def _split_excess_waits(bir_json: bytes) -> bytes:
    """The walrus in this container rejects instructions carrying more than
    one sem-wait ("Too many sync wait commands"). Hoist all but the last
    wait of any instruction into single-wait EventSemaphore instructions
    inserted just before it on the same engine (same-engine program order
    makes this semantically identical)."""
    import json

    m = json.loads(bir_json)
    for f in m["functions"]:
        for bb in f["blocks"]:
            out_insts = []
            for ins in bb["instructions"]:
                si = ins.get("sync_info")
                waits = (si or {}).get("on_wait") or []
                if len(waits) > 1:
                    for i, w in enumerate(waits[:-1]):
                        out_insts.append(
                            {
                                "debug": ins.get("debug", 0),
                                "engine": ins["engine"],
                                "ins": [],
                                "name": f"{ins['name']}_sw{i}",
                                "opcode": "EventSemaphore",
                                "outs": [],
                                "sync_info": {"on_update": [], "on_wait": [w]},
                            }
                        )
                    si["on_wait"] = [waits[-1]]
                out_insts.append(ins)
            bb["instructions"] = out_insts
    return json.dumps(m).encode()


def _patch_compile_for_wait_limit():
    import concourse.bass_utils as bu
    import concourse.bass2jax as b2j

    if getattr(bu, "_wait_split_patched", False):
        return
    orig = bu.compile_bir_kernel

    def compile_bir_kernel(bir_json, tmpdir, neff_name="file.neff"):
        return orig(_split_excess_waits(bir_json), tmpdir, neff_name)

    bu.compile_bir_kernel = compile_bir_kernel
    b2j.compile_bir_kernel = compile_bir_kernel
    bu._wait_split_patched = True


def _install_ntff_hook_shim():
    """The trimmed image lacks antenv.axon_hooks; recreate it so
    run_bass_kernel_spmd(trace=True) can capture NTFF profiles via axon."""
    if "antenv.axon_hooks" in sys.modules:
        return
    try:
        import antenv
        from trn_agent_boot.trn_boot import _ntff_profile_via_ctypes
    except Exception:
        return
    mod = types.ModuleType("antenv.axon_hooks")
    _hook = _ntff_profile_via_ctypes("/opt/axon/libaxon_pjrt.so")
    mod.get_axon_ntff_profile_hook = lambda: _hook
    mod.set_axon_ntff_profile_hook = lambda h: None
    sys.modules["antenv.axon_hooks"] = mod
    antenv.axon_hooks = mod


def build_kernel() -> bass.Bass:
    nc = bass.Bass(target_bir_lowering=False, trn_type="TRN2")
    # full augmented x^T in DoubleRow pair layout: [pair, part, slab, col]
    xt = nc.dram_tensor("xt", [NPAIR, P, 2, B], FP8, kind="ExternalInput")
    # stationary side: same restricted to this core's rows, bias rows -> 1.0
    xtl = nc.dram_tensor("xtl", [NPAIR, P, 2, BL], FP8, kind="ExternalInput")
    out = nc.dram_tensor("out", [P, NI * 8], F32, kind="ExternalOutput")

    with TileContext(nc) as tc:
        with (
            tc.tile_pool(name="xtp", bufs=1) as xt_pool,
            tc.tile_pool(name="cnd", bufs=2) as cand_pool,
            tc.tile_pool(name="res", bufs=1) as res_pool,
            tc.tile_pool(name="ps", bufs=8, space="PSUM") as psum_pool,
        ):
            xtl_sb = []
            for pr in range(NPAIR):
                t = xt_pool.tile([P, 2, BL], FP8, name=f"xtl{pr}")
                xtl_sb.append(t)
            nc.gpsimd.dma_start(xtl_sb[0], xtl[0])
            nc.gpsimd.dma_start(xtl_sb[1], xtl[1])
            nc.gpsimd.dma_start(xtl_sb[2], xtl[2])

            # big moving tiles in column quarters; the first quarter of each
            # pair is issued from a different engine so the three issues (and
            # transfers) run concurrently and compute can start early
            xt_sb = [
                xt_pool.tile([P, 2, B], FP8, name=f"xt{pr}") for pr in range(NPAIR)
            ]
            NQD = 4
            H = B // NQD
            issue_engines = [nc.sync, nc.scalar, nc.gpsimd]
            for pr in range(NPAIR):
                issue_engines[pr].dma_start(
                    xt_sb[pr][:, :, 0:H], xt[pr][:, :, 0:H]
                )
            for h in range(1, NQD):
                for pr in range(NPAIR):
                    nc.sync.dma_start(
                        xt_sb[pr][:, :, h * H : (h + 1) * H],
                        xt[pr][:, :, h * H : (h + 1) * H],
                    )

            # PE clock warmup during the DMA: cheap 16-col-stationary matmuls
            # (one tile reconfig before the main sweep, none inside it)
            wu = psum_pool.tile([P, 512], F32, name="ps")
            for w in range(16):
                nc.tensor.matmul(
                    wu[:, 0:8],
                    lhsT=xtl_sb[0][:, :, 0:P],
                    rhs=xtl_sb[0][:, :, 0:8],
                    start=True,
                    stop=True,
                    perf_mode=DR,
                )

            # ---- main sweep ----
            # Chunks 0-7 of each row tile: DVE MAX8 straight off the PSUM
            # bank. Chunks 8-15: scalar evacuates the bank to an SBUF strip;
            # DVE then takes top-8 of each 4-chunk strip in one MAX8. This
            # balances DVE (~10.1us/row tile) against PE (~10.4us). The last
            # row tile stays all-direct so the tail ends with a short MAX8.
            # quarter-major order: all row tiles consume DMA quarter q before
            # any chunk of quarter q+1, so PE has 20us of work per ~13us of
            # remaining DMA and never stalls on the stream-in
            top_all = res_pool.tile([P, NI * 8], F32, name="top_all")
            cands = {
                i: cand_pool.tile([P, NJ * 8], F32, name=f"cand{i}")
                for i in range(NI)
            }
            JQ = NJ // NQD  # 4 chunks per quarter
            for q in range(NQD):
                for i in range(NI):
                    for jq in range(JQ):
                        j = q * JQ + jq
                        ps = psum_pool.tile([P, 512], F32, name="ps")
                        for pr in range(NPAIR):
                            nc.tensor.matmul(
                                ps,
                                lhsT=xtl_sb[pr][:, :, i * P : (i + 1) * P],
                                rhs=xt_sb[pr][:, :, j * 512 : (j + 1) * 512],
                                start=(pr == 0),
                                stop=(pr == NPAIR - 1),
                                perf_mode=DR,
                            )
                        nc.vector.max(
                            out=cands[i][:, j * 8 : (j + 1) * 8], in_=ps
                        )
                        # deferred merges during the last quarter
                        if q == NQD - 1 and i > 0 and jq == 2:
                            nc.vector.max(
                                out=top_all[:, (i - 1) * 8 : i * 8],
                                in_=cands[i - 1],
                            )
            nc.vector.max(out=top_all[:, (NI - 1) * 8 :], in_=cands[NI - 1])
            nc.sync.dma_start(out[:], top_all)

    return nc


def _prep_inputs(x: np.ndarray):
    f8 = ml_dtypes.float8_e4m3
    sq = np.sum(x.astype(np.float64) * x.astype(np.float64), axis=1)
    C = float(sq.mean())
    bias_full = ((C - sq) / 2.0).astype(np.float32)
    bias_hi = bias_full.astype(f8)
    bias_lo = (bias_full - bias_hi.astype(np.float32)).astype(f8)
    # augmented moving operand: rows 0..765 = x dims, 766/767 = bias hi/lo
    xaug = np.empty((D, B), f8)
    xaug[: D - 2] = x.T[: D - 2].astype(f8)
    xaug[D - 2] = bias_hi
    xaug[D - 1] = bias_lo
    # stationary variant: bias rows replaced by 1.0
    xaug_st = xaug.copy()
    xaug_st[D - 2 :] = np.float32(1.0).astype(f8)
    # [768, 8192] -> [pair, part, slab, col]
    xt_dr = np.ascontiguousarray(
        xaug.reshape(NPAIR, 2, P, B).transpose(0, 2, 1, 3)
    )
    xt_dr_st = np.ascontiguousarray(
        xaug_st.reshape(NPAIR, 2, P, B).transpose(0, 2, 1, 3)
    )
    sq32 = (sq + C).astype(np.float32)
    return xt_dr, xt_dr_st, sq32


def run(inputs: dict, trace: bool = False):
    _patch_compile_for_wait_limit()
    if trace:
        _install_ntff_hook_shim()

    x = np.asarray(inputs["student_output"], dtype=np.float32)
    assert x.shape == (B, D), x.shape
    xt_dr, xt_dr_st, sq32 = _prep_inputs(x)

    nc = build_kernel()
    in_maps = []
    for c in range(NCORES):
        r0 = c * BL
        in_maps.append(
            {
                "xt": xt_dr,
                "xtl": np.ascontiguousarray(xt_dr_st[:, :, :, r0 : r0 + BL]),
            }
        )
    res = run_bass_kernel_spmd(
        nc, in_maps, core_ids=list(range(NCORES)), trace=trace
    )
    # host epilogue: top8 v values -> distances -> loss
    total = 0.0
    for c in range(NCORES):
        top8 = res.results[c]["out"].astype(np.float64)  # [P, NI*8]
        v5 = top8.reshape(P, NI, 8)[:, :, 1 : K + 1]     # drop self, keep 5 NN
        sqc = sq32[c * BL : (c + 1) * BL].reshape(NI, P).T  # [P, NI]
        d5 = np.sqrt(np.maximum(sqc[:, :, None] - 2.0 * v5, 0.0))
        total += np.log(d5.mean(axis=2) + EPS).sum()
    loss = np.float32(-total / B)
    return np.asarray(loss, dtype=np.float32), res


def kernel(**inputs) -> np.ndarray:
    out, _ = run(inputs, trace=False)
    return out


# revision 27
# speedup vs baseline: 1.0443x; 1.0092x over previous
"""KNN entropy loss (k=5, B=8192, D=768) on 8 TRN2 NeuronCores.

Sharding: rows of x split 1024/core. Each core computes its [1024 x 8192]
block of v[i,j] = sum_{k<766} x_ik x_jk + (C - ||x_j||^2)/2 with fp8e4
DoubleRow matmuls (3 uniform pairs per 512-col chunk, f32 PSUM). The two
bias values ride as augmented contraction rows 766/767 (fp8 hi/lo split),
with 1.0 in those rows on the stationary side, so every chunk is exactly
3 equal-shape matmuls — no PE tile reconfiguration anywhere.

argmax_j v = argmin_j d^2, so a DVE MAX8 straight off each PSUM bank
yields per-chunk top-8 candidates; a second MAX8 merges the 16 chunk
results per row tile. The merged top-8 v values are DMA'd out; the host
reconstructs d = sqrt((||x_i||^2 + C) - 2 v) for ranks 1..5 (rank 0 is
the self-match), then loss = -mean(log(mean_k d + eps)).

All norms/layout prep happens on the host (like the baseline's
transpose/cast prep). Norms use the full 768 dims in f64; only the two
cross-term dims 766/767 of the gram are absorbed by the bias rows, which
perturbs d^2 by ~0.2% rms — far inside the 2e-2 tolerance (measured
rel err ~5e-5).
"""

import sys
import types

import numpy as np
import ml_dtypes

import concourse.bass as bass
import concourse.mybir as mybir
from concourse.tile import TileContext
from concourse.bass_utils import run_bass_kernel_spmd

P = 128
B = 8192
D = 768
NCORES = 8
BL = B // NCORES          # 1024 local rows per core
NPAIR = D // 256          # 3 DoubleRow contraction pairs
NI = BL // P              # 8 row tiles per core
NJ = B // 512             # 16 column chunks of 512
K = 5
EPS = 1e-8

FP8 = mybir.dt.float8e4
F32 = mybir.dt.float32
DR = mybir.MatmulPerfMode.DoubleRow


def _split_excess_waits(bir_json: bytes) -> bytes:
    """The walrus in this container rejects instructions carrying more than
    one sem-wait ("Too many sync wait commands"). Hoist all but the last
    wait of any instruction into single-wait EventSemaphore instructions
    inserted just before it on the same engine (same-engine program order
    makes this semantically identical)."""
    import json

    m = json.loads(bir_json)
    for f in m["functions"]:
        for bb in f["blocks"]:
            out_insts = []
            for ins in bb["instructions"]:
                si = ins.get("sync_info")
                waits = (si or {}).get("on_wait") or []
                if len(waits) > 1:
                    for i, w in enumerate(waits[:-1]):
                        out_insts.append(
                            {
                                "debug": ins.get("debug", 0),
                                "engine": ins["engine"],
                                "ins": [],
                                "name": f"{ins['name']}_sw{i}",
                                "opcode": "EventSemaphore",
                                "outs": [],
                                "sync_info": {"on_update": [], "on_wait": [w]},
                            }
                        )
                    si["on_wait"] = [waits[-1]]
                out_insts.append(ins)
            bb["instructions"] = out_insts
    return json.dumps(m).encode()


def _patch_compile_for_wait_limit():
    import concourse.bass_utils as bu
    import concourse.bass2jax as b2j

    if getattr(bu, "_wait_split_patched", False):
        return
    orig = bu.compile_bir_kernel

    def compile_bir_kernel(bir_json, tmpdir, neff_name="file.neff"):
        return orig(_split_excess_waits(bir_json), tmpdir, neff_name)

    bu.compile_bir_kernel = compile_bir_kernel
    b2j.compile_bir_kernel = compile_bir_kernel
    bu._wait_split_patched = True


def _install_ntff_hook_shim():
    """The trimmed image lacks antenv.axon_hooks; recreate it so
    run_bass_kernel_spmd(trace=True) can capture NTFF profiles via axon."""
    if "antenv.axon_hooks" in sys.modules:
        return
    try:
        import antenv
        from trn_agent_boot.trn_boot import _ntff_profile_via_ctypes
    except Exception:
        return
    mod = types.ModuleType("antenv.axon_hooks")
    _hook = _ntff_profile_via_ctypes("/opt/axon/libaxon_pjrt.so")
    mod.get_axon_ntff_profile_hook = lambda: _hook
    mod.set_axon_ntff_profile_hook = lambda h: None
    sys.modules["antenv.axon_hooks"] = mod
    antenv.axon_hooks = mod


def build_kernel() -> bass.Bass:
    nc = bass.Bass(target_bir_lowering=False, trn_type="TRN2")
    # full augmented x^T in DoubleRow pair layout: [pair, part, slab, col]
    xt = nc.dram_tensor("xt", [NPAIR, P, 2, B], FP8, kind="ExternalInput")
    # stationary side: same restricted to this core's rows, bias rows -> 1.0
    xtl = nc.dram_tensor("xtl", [NPAIR, P, 2, BL], FP8, kind="ExternalInput")
    out = nc.dram_tensor("out", [P, NI * 8], F32, kind="ExternalOutput")

    with TileContext(nc) as tc:
        with (
            tc.tile_pool(name="xtp", bufs=1) as xt_pool,
            tc.tile_pool(name="cnd", bufs=2) as cand_pool,
            tc.tile_pool(name="res", bufs=1) as res_pool,
            tc.tile_pool(name="ps", bufs=8, space="PSUM") as psum_pool,
        ):
            xtl_sb = []
            for pr in range(NPAIR):
                t = xt_pool.tile([P, 2, BL], FP8, name=f"xtl{pr}")
                xtl_sb.append(t)
            nc.gpsimd.dma_start(xtl_sb[0], xtl[0])
            nc.gpsimd.dma_start(xtl_sb[1], xtl[1])
            nc.gpsimd.dma_start(xtl_sb[2], xtl[2])

            # big moving tiles in column quarters; the first quarter of each
            # pair is issued from a different engine so the three issues (and
            # transfers) run concurrently and compute can start early
            xt_sb = [
                xt_pool.tile([P, 2, B], FP8, name=f"xt{pr}") for pr in range(NPAIR)
            ]
            NQD = 4
            H = B // NQD
            issue_engines = [nc.sync, nc.scalar, nc.scalar]
            for pr in range(NPAIR):
                issue_engines[pr].dma_start(
                    xt_sb[pr][:, :, 0:H], xt[pr][:, :, 0:H]
                )
            for h in range(1, NQD):
                for pr in range(NPAIR):
                    nc.sync.dma_start(
                        xt_sb[pr][:, :, h * H : (h + 1) * H],
                        xt[pr][:, :, h * H : (h + 1) * H],
                    )

            # PE clock warmup during the DMA: cheap 16-col-stationary matmuls
            # (one tile reconfig before the main sweep, none inside it)
            wu = psum_pool.tile([P, 512], F32, name="ps")
            for w in range(16):
                nc.tensor.matmul(
                    wu[:, 0:8],
                    lhsT=xtl_sb[0][:, :, 0:P],
                    rhs=xtl_sb[0][:, :, 0:8],
                    start=True,
                    stop=True,
                    perf_mode=DR,
                )

            # ---- main sweep ----
            # Chunks 0-7 of each row tile: DVE MAX8 straight off the PSUM
            # bank. Chunks 8-15: scalar evacuates the bank to an SBUF strip;
            # DVE then takes top-8 of each 4-chunk strip in one MAX8. This
            # balances DVE (~10.1us/row tile) against PE (~10.4us). The last
            # row tile stays all-direct so the tail ends with a short MAX8.
            # quarter-major order: all row tiles consume DMA quarter q before
            # any chunk of quarter q+1, so PE has 20us of work per ~13us of
            # remaining DMA and never stalls on the stream-in
            top_all = res_pool.tile([P, NI * 8], F32, name="top_all")
            cands = {
                i: cand_pool.tile([P, NJ * 8], F32, name=f"cand{i}")
                for i in range(NI)
            }
            JQ = NJ // NQD  # 4 chunks per quarter
            for q in range(NQD):
                for i in range(NI):
                    for jq in range(JQ):
                        j = q * JQ + jq
                        ps = psum_pool.tile([P, 512], F32, name="ps")
                        for pr in range(NPAIR):
                            nc.tensor.matmul(
                                ps,
                                lhsT=xtl_sb[pr][:, :, i * P : (i + 1) * P],
                                rhs=xt_sb[pr][:, :, j * 512 : (j + 1) * 512],
                                start=(pr == 0),
                                stop=(pr == NPAIR - 1),
                                perf_mode=DR,
                            )
                        nc.vector.max(
                            out=cands[i][:, j * 8 : (j + 1) * 8], in_=ps
                        )
                        # deferred merges during the last quarter
                        if q == NQD - 1 and i > 0 and jq == 2:
                            nc.vector.max(
                                out=top_all[:, (i - 1) * 8 : i * 8],
                                in_=cands[i - 1],
                            )
            nc.vector.max(out=top_all[:, (NI - 1) * 8 :], in_=cands[NI - 1])
            nc.sync.dma_start(out[:], top_all)

    return nc


def _prep_inputs(x: np.ndarray):
    f8 = ml_dtypes.float8_e4m3
    sq = np.sum(x.astype(np.float64) * x.astype(np.float64), axis=1)
    C = float(sq.mean())
    bias_full = ((C - sq) / 2.0).astype(np.float32)
    bias_hi = bias_full.astype(f8)
    bias_lo = (bias_full - bias_hi.astype(np.float32)).astype(f8)
    # augmented moving operand: rows 0..765 = x dims, 766/767 = bias hi/lo
    xaug = np.empty((D, B), f8)
    xaug[: D - 2] = x.T[: D - 2].astype(f8)
    xaug[D - 2] = bias_hi
    xaug[D - 1] = bias_lo
    # stationary variant: bias rows replaced by 1.0
    xaug_st = xaug.copy()
    xaug_st[D - 2 :] = np.float32(1.0).astype(f8)
    # [768, 8192] -> [pair, part, slab, col]
    xt_dr = np.ascontiguousarray(
        xaug.reshape(NPAIR, 2, P, B).transpose(0, 2, 1, 3)
    )
    xt_dr_st = np.ascontiguousarray(
        xaug_st.reshape(NPAIR, 2, P, B).transpose(0, 2, 1, 3)
    )
    sq32 = (sq + C).astype(np.float32)
    return xt_dr, xt_dr_st, sq32


def run(inputs: dict, trace: bool = False):
    _patch_compile_for_wait_limit()
    if trace:
        _install_ntff_hook_shim()

    x = np.asarray(inputs["student_output"], dtype=np.float32)
    assert x.shape == (B, D), x.shape
    xt_dr, xt_dr_st, sq32 = _prep_inputs(x)

    nc = build_kernel()
    in_maps = []
    for c in range(NCORES):
        r0 = c * BL
        in_maps.append(
            {
                "xt": xt_dr,
                "xtl": np.ascontiguousarray(xt_dr_st[:, :, :, r0 : r0 + BL]),
            }
        )
    res = run_bass_kernel_spmd(
        nc, in_maps, core_ids=list(range(NCORES)), trace=trace
    )
    # host epilogue: top8 v values -> distances -> loss
    total = 0.0
    for c in range(NCORES):
        top8 = res.results[c]["out"].astype(np.float64)  # [P, NI*8]
        v5 = top8.reshape(P, NI, 8)[:, :, 1 : K + 1]     # drop self, keep 5 NN
        sqc = sq32[c * BL : (c + 1) * BL].reshape(NI, P).T  # [P, NI]
        d5 = np.sqrt(np.maximum(sqc[:, :, None] - 2.0 * v5, 0.0))
        total += np.log(d5.mean(axis=2) + EPS).sum()
    loss = np.float32(-total / B)
    return np.asarray(loss, dtype=np.float32), res


def kernel(**inputs) -> np.ndarray:
    out, _ = run(inputs, trace=False)
    return out
